# revision 39
# baseline (speedup 1.0000x reference)
"""Trainium2 Bass kernel for nn_Net_420906795534 (GNN: 3x GraphConv + TopKPooling + readout + MLP).

Sharding: data-parallel over graphs - 8 graphs per NeuronCore x 8 cores.
Host does index-only preprocessing: per-graph dense adjacency count matrices
(f32, exact) and layout reshapes. All float compute runs on device.

Device algorithm (v2, f32r pipeline):
  All heavy matmuls use float32r operands (FP22 reads, 1 cycle/row for
  moving dims >= 256) - no bf16 hi/lo split needed. Per layer:
    conv:    agg_T[f,d] = sum_c h_nm_c(f32r).T @ A_c(f32r)   (PE)
             h_T = relu(W_rel.T @ agg_T + W_root.T @ hTs + b) (PE f32r + ACT)
    pool:    u = (h.w)/||w||; exact jax.lax.top_k replication via the
             clip-at-XSAT lexicographic cascade (gpsimd kth_largest),
             batched across a PAIR of graphs ([128,16] tiles).
    readout: masked max in fp16 (DVE reduce + PE transpose), mean via
             ones-matmuls accumulated in PSUM.
  Two pair-chains of graphs are software-pipelined (generator interleave)
  so PE stays busy through the selection cascade; A tiles (f32) stream
  per graph with dst-half split DMAs.
"""
import sys
sys.path.insert(0, '/opt/trn_rl_repo')
import math
import numpy as np

B_GRAPHS, N, DEG = 64, 1024, 16
IN_F, HID = 20, 128
G_PER_CORE = 8
N_CORES = 8
P = 128
NCH = N // P  # 8 node chunks per graph
XSAT = np.float32(7.998811721801758)  # XLA-cpu f32 tanh saturation cutoff
K1, K2, K3 = 820, 656, 525           # ceil(0.8*n) chain
NDROP = {1: N - K1, 2: K1 - K2, 3: K2 - K3}      # 204, 164, 131
NVALID = {1: N, 2: K1, 3: K2}
KKEEP = {1: K1, 2: K2, 3: K3}
BIG = 1e20
INVALID = -1e30


def _quantile_for_rank(rank_m2: int, n_valid: int) -> float:
    """Return q so kth_largest's k_adj == rank_m2 exactly."""
    lo = int(math.ceil(rank_m2 * (1 << 32) / (n_valid - 1)))
    hi = int(math.ceil((rank_m2 + 1) * (1 << 32) / (n_valid - 1))) - 1
    omq = (lo + hi) // 2
    assert (omq * (n_valid - 1)) >> 32 == rank_m2
    return 1.0 - omq / (1 << 32)


def build_program():
    import concourse.bacc as bacc
    import concourse.mybir as mybir
    import concourse.tile as tile
    from concourse.masks import make_identity

    f32 = mybir.dt.float32
    f32r = mybir.dt.float32r
    f16 = mybir.dt.float16
    i32 = mybir.dt.int32
    AF = mybir.ActivationFunctionType
    ALU = mybir.AluOpType
    AX = mybir.AxisListType

    nc = bacc.Bacc("TRN2", target_bir_lowering=False, debug=False,
                   num_devices=N_CORES)

    # ---------------- DRAM I/O ----------------
    d_x = nc.dram_tensor("x_nm", [P, G_PER_CORE * NCH * IN_F], f32r,
                         kind="ExternalInput")
    d_xT = nc.dram_tensor("xT", [G_PER_CORE, IN_F, N], f32r,
                          kind="ExternalInput")
    d_A = nc.dram_tensor("A_sd", [G_PER_CORE, P, NCH * N], f32r,
                         kind="ExternalInput")
    d_w = {}
    for l, infl in ((1, IN_F), (2, HID), (3, HID)):
        d_w[f"W_rel{l}"] = nc.dram_tensor(f"W_rel{l}", [infl, HID], f32r,
                                          kind="ExternalInput")
        d_w[f"W_root{l}"] = nc.dram_tensor(f"W_root{l}", [infl, HID], f32r,
                                           kind="ExternalInput")
        d_w[f"b_rel{l}"] = nc.dram_tensor(f"b_rel{l}", [HID, 1], f32,
                                          kind="ExternalInput")
        d_w[f"w_pool{l}"] = nc.dram_tensor(f"w_pool{l}", [HID, 1], f32r,
                                           kind="ExternalInput")
    d_w["W_lin1a"] = nc.dram_tensor("W_lin1a", [HID, HID], f32, kind="ExternalInput")
    d_w["W_lin1b"] = nc.dram_tensor("W_lin1b", [HID, HID], f32, kind="ExternalInput")
    d_w["b_lin1"] = nc.dram_tensor("b_lin1", [HID, 1], f32, kind="ExternalInput")
    d_w["W_lin2"] = nc.dram_tensor("W_lin2", [HID, 64], f32, kind="ExternalInput")
    d_w["b_lin2"] = nc.dram_tensor("b_lin2", [64, 1], f32, kind="ExternalInput")
    d_w["W_lin3"] = nc.dram_tensor("W_lin3", [64, 1], f32, kind="ExternalInput")
    d_w["b_lin3"] = nc.dram_tensor("b_lin3", [1, 1], f32, kind="ExternalInput")
    d_out = nc.dram_tensor("out", [1, G_PER_CORE], f32, kind="ExternalOutput")

    HN = NCH * HID  # 1024
    HALF = 512

    with tile.TileContext(nc) as tc:
        with (
            tc.tile_pool(name="const", bufs=1) as cpool,
            tc.tile_pool(name="apool", bufs=4) as apool,
            tc.tile_pool(name="xtpool", bufs=2) as xtpool,
            tc.tile_pool(name="htpool", bufs=3) as htpool,
            tc.tile_pool(name="hppool", bufs=1) as hppool,
            tc.tile_pool(name="hspool", bufs=1) as hspool,
            tc.tile_pool(name="aggpool", bufs=2) as aggpool,
            tc.tile_pool(name="hmpool", bufs=2) as hmpool,
            tc.tile_pool(name="selpool", bufs=2) as selpool,
            tc.tile_pool(name="psA", bufs=2, space="PSUM") as psA,
            tc.tile_pool(name="psH", bufs=2, space="PSUM") as psH,
            tc.tile_pool(name="psT", bufs=2, space="PSUM") as psT,
            tc.tile_pool(name="psS", bufs=2, space="PSUM") as psS,
        ):
            # ---------- constants / weights ----------
            ident_f = cpool.tile([P, P], f32)
            make_identity(nc, ident_f[:])
            ident_r = cpool.tile([P, P], f32r)
            nc.scalar.copy(ident_r[:], ident_f[:])
            ident_h = cpool.tile([P, P], f16)
            nc.scalar.copy(ident_h[:], ident_f[:])
            ones_f = cpool.tile([P, 1], f32)
            nc.vector.memset(ones_f[:], 1.0)
            # node index (p + 128*c) replicated for both graphs of a pair
            idx1 = cpool.tile([P, NCH], f32)
            idx1_i = cpool.tile([P, NCH], i32)
            nc.gpsimd.iota(idx1_i[:], pattern=[[128, NCH]], base=0,
                           channel_multiplier=1)
            nc.vector.tensor_copy(idx1[:], idx1_i[:])
            idx_pair = cpool.tile([P, 2 * NCH], f32)
            for i in range(2):
                nc.vector.tensor_copy(idx_pair[:, i * NCH:(i + 1) * NCH], idx1[:])

            w_t = {}
            for name, dd in d_w.items():
                t = cpool.tile(list(dd.shape), dd.dtype, tag=name)
                nc.sync.dma_start(out=t[:], in_=dd[:])
                w_t[name] = t

            # x node-major (all graphs), f32r
            x_nm = cpool.tile([P, G_PER_CORE * NCH * IN_F], f32r)
            nc.sync.dma_start(out=x_nm[:], in_=d_x[:])

            # invnorm_l = 1/||w_pool_l|| replicated [P,1]
            invnorm = {}
            for l in (1, 2, 3):
                pnw = psS.tile([1, 1], f32, tag="s")
                nc.tensor.matmul(pnw[:], lhsT=w_t[f"w_pool{l}"][:].bitcast(f32),
                                 rhs=w_t[f"w_pool{l}"][:].bitcast(f32),
                                 start=True, stop=True)
                nrm = selpool.tile([1, 1], f32, tag="nrm")
                nc.scalar.activation(nrm[:], pnw[:], AF.Sqrt)
                inv = selpool.tile([1, 1], f32, tag="inv")
                nc.vector.reciprocal(inv[:], nrm[:])
                invr = cpool.tile([P, 1], f32, tag=f"invn{l}")
                nc.gpsimd.partition_broadcast(invr[:], inv[:], channels=P)
                invnorm[l] = invr

            # global readout accumulators [feat, graph]
            zmax = cpool.tile([P, G_PER_CORE], f32)
            zmean = cpool.tile([P, G_PER_CORE], f32)
            nc.vector.memset(zmax[:], 0.0)
            nc.vector.memset(zmean[:], 0.0)

            # ---------- A tile management (bufs=4 rotation) ----------
            A_t = {}
            xT_t = {}

            def load_xT(g):
                t = xtpool.tile([IN_F, N], f32r, tag="xT", name=f"xT{g}")
                nc.sync.dma_start(out=t[:], in_=d_xT[g])
                xT_t[g] = t

            def load_A(g):
                # A laid out [p, (half, chunk, 512)]: four contiguous-quarter
                # DMAs so agg matmuls start as soon as their slice lands
                t = apool.tile([P, NCH * N], f32r, tag="A", name=f"A{g}")
                Q = NCH * N // 4
                for q in range(4):
                    nc.sync.dma_start(out=t[:, q * Q:(q + 1) * Q],
                                      in_=d_A[g][:, q * Q:(q + 1) * Q])
                A_t[g] = t

            # per-graph state tiles (rotate via per-slot tags)
            hp32 = {}   # node-major scaled h' (f32r), agg lhsT of next layer
            hT_s = {}   # feature-major scaled h' (f32r), root rhs of next layer
            hT_new = {}  # feature-major unscaled h (f32r), transient per layer

            def phaseA(g, l, pz_pair, i):
                """conv + linear + relu + scores for graph g, layer l."""
                infl = IN_F if l == 1 else HID
                At = A_t[g]
                hTs_src = xT_t[g] if l == 1 else hT_s[g]
                aggT = aggpool.tile([infl, N], f32r, tag="aggT", name=f"aggT{g}_{l}")
                ht = htpool.tile([HID, N], f32r, tag="hT", name=f"hT{g}_{l}")
                for half in range(2):
                    sl = slice(half * HALF, (half + 1) * HALF)
                    pagg = psA.tile([infl, HALF], f32, tag="agg")
                    for c in range(NCH):
                        if l == 1:
                            lhs = x_nm[:, (g * NCH + c) * IN_F:(g * NCH + c + 1) * IN_F]
                        else:
                            lhs = hp32[g][:, c * HID:(c + 1) * HID]
                        nc.tensor.matmul(
                            pagg[:], lhsT=lhs,
                            rhs=At[:, (half * NCH + c) * HALF:(half * NCH + c + 1) * HALF],
                            start=(c == 0), stop=(c == NCH - 1),
                            skip_group_check=True)
                    nc.scalar.copy(aggT[:, sl], pagg[:])
                    ph = psH.tile([HID, HALF], f32, tag="ph")
                    nc.tensor.matmul(ph[:], lhsT=w_t[f"W_root{l}"][:],
                                     rhs=hTs_src[:, sl], start=True, stop=False,
                                     skip_group_check=True)
                    nc.tensor.matmul(ph[:], lhsT=w_t[f"W_rel{l}"][:],
                                     rhs=aggT[:, sl], start=False, stop=True,
                                     skip_group_check=True)
                    nc.scalar.activation(ht[:, sl], ph[:], AF.Relu,
                                         bias=w_t[f"b_rel{l}"][:, 0:1])
                hT_new[g] = ht
                # scores: pz[:, i*8+c] = h_chunk.T @ w_pool
                for c in range(NCH):
                    nc.tensor.matmul(
                        pz_pair[:, i * NCH + c:i * NCH + c + 1],
                        lhsT=ht[:, c * P:(c + 1) * P].bitcast(f32),
                        rhs=w_t[f"w_pool{l}"][:].bitcast(f32),
                        start=(c == 0), stop=(c == NCH - 1),
                        skip_group_check=True)

            def selection(pr, l, pz_pair, keep, ucs):
                """Batched pair top-k keep mask. Returns (keep_new, sk, maskadd)."""
                W = 2 * NCH
                nvalid, ndrop = NVALID[l], NDROP[l]
                u = selpool.tile([P, W], f32, tag="u")
                nc.scalar.activation(u[:], pz_pair[:], AF.Copy,
                                     scale=invnorm[l][:, 0:1])
                uc = selpool.tile([P, W], f32, tag=f"uc{l}")
                nc.vector.tensor_scalar(out=uc[:], in0=u[:], scalar1=float(XSAT),
                                        scalar2=float(-XSAT), op0=ALU.min,
                                        op1=ALU.max)
                ucs.append(uc)

                comps = [("u", t) for t in reversed(ucs)] + [("i", idx_pair)]
                bg = selpool.tile([P, W], f32, tag="bg")
                nc.vector.tensor_scalar(out=bg[:], in0=keep[:],
                                        scalar1=float(-INVALID),
                                        scalar2=float(INVALID),
                                        op0=ALU.mult, op1=ALU.add)
                ic = selpool.tile([P, W], f32, tag="ic")
                nc.vector.tensor_copy(ic[:], keep[:])
                dropped = selpool.tile([P, W], f32, tag="dropped")
                nc.vector.memset(dropped[:], 0.0)
                q = _quantile_for_rank(ndrop - 2, nvalid)
                for j, (kind, comp) in enumerate(comps):
                    key = selpool.tile([P, W], f32, tag="key")
                    nc.vector.tensor_tensor(out=key[:], in0=comp[:], in1=ic[:],
                                            op=ALU.mult)
                    if kind == "u":
                        nc.vector.scalar_tensor_tensor(
                            out=key[:], in0=key[:], scalar=-1.0, in1=bg[:],
                            op0=ALU.mult, op1=ALU.add)
                    else:
                        nc.vector.tensor_tensor(out=key[:], in0=key[:],
                                                in1=bg[:], op=ALU.add)
                    tv = selpool.tile([1, 4], f32, tag="tv")
                    for i in range(2):
                        nc.gpsimd.kth_largest(
                            tv[:, 2 * i:2 * i + 2],
                            key[:, i * NCH:(i + 1) * NCH],
                            n_per_lane=NCH, k=ndrop, quantile=q)
                    vrep = selpool.tile([P, 4], f32, tag="vrep")
                    nc.gpsimd.partition_broadcast(vrep[:], tv[:], channels=P)
                    v3d = vrep[:, 1::2].rearrange(
                        "p (g o) -> p g o", o=1).to_broadcast([P, 2, NCH])
                    last = (j == len(comps) - 1)
                    nd = selpool.tile([P, W], f32, tag="nd")
                    nc.vector.tensor_tensor(
                        out=nd[:].rearrange("p (g c) -> p g c", g=2),
                        in0=key[:].rearrange("p (g c) -> p g c", g=2),
                        in1=v3d, op=(ALU.is_ge if last else ALU.is_gt))
                    nc.vector.tensor_tensor(out=nd[:], in0=nd[:], in1=ic[:],
                                            op=ALU.mult)
                    nc.vector.tensor_tensor(out=dropped[:], in0=dropped[:],
                                            in1=nd[:], op=ALU.add)
                    if not last:
                        eq = selpool.tile([P, W], f32, tag="eq")
                        nc.vector.tensor_tensor(
                            out=eq[:].rearrange("p (g c) -> p g c", g=2),
                            in0=key[:].rearrange("p (g c) -> p g c", g=2),
                            in1=v3d, op=ALU.is_equal)
                        ic_new = selpool.tile([P, W], f32, tag="ic")
                        nc.vector.tensor_tensor(out=ic_new[:], in0=eq[:],
                                                in1=ic[:], op=ALU.mult)
                        safe = selpool.tile([P, W], f32, tag="safe")
                        nc.vector.tensor_tensor(out=safe[:], in0=ic[:],
                                                in1=ic_new[:], op=ALU.subtract)
                        nc.vector.tensor_tensor(out=safe[:], in0=safe[:],
                                                in1=nd[:], op=ALU.subtract)
                        nc.vector.scalar_tensor_tensor(
                            out=bg[:], in0=nd[:], scalar=float(BIG),
                            in1=bg[:], op0=ALU.mult, op1=ALU.add)
                        nc.vector.scalar_tensor_tensor(
                            out=bg[:], in0=safe[:], scalar=float(-BIG),
                            in1=bg[:], op0=ALU.mult, op1=ALU.add)
                        ic = ic_new
                keep_new = selpool.tile([P, W], f32, tag="keep", bufs=4,
                                        name=f"keep{pr}_{l}")
                nc.vector.tensor_tensor(out=keep_new[:], in0=keep[:],
                                        in1=dropped[:], op=ALU.subtract)
                s = selpool.tile([P, W], f32, tag="s")
                nc.scalar.activation(s[:], u[:], AF.Tanh)
                sk = selpool.tile([P, W], f32, tag="sk")
                nc.vector.tensor_tensor(out=sk[:], in0=s[:], in1=keep_new[:],
                                        op=ALU.mult)
                # mask offset finite in fp16 (-inf would trip finite checks)
                maskadd = selpool.tile([P, W], f32, tag="maskadd")
                nc.vector.tensor_scalar(out=maskadd[:], in0=keep_new[:],
                                        scalar1=60000.0, scalar2=-60000.0,
                                        op0=ALU.mult, op1=ALU.add)
                return keep_new, sk, maskadd

            def phaseB1(g, l, i, sk):
                """Transposes of h_T and scaled node-major h' (hp)."""
                ht = hT_new[g]
                hp = hppool.tile([P, HN], f32r, tag=f"hp{g % 4}",
                                 name=f"hp{g}_{l}")
                for hh in range(2):
                    pt = psT.tile([P, HALF], f32r, tag="pt", name=f"pt{g}_{l}_{hh}")
                    for c in range(4):
                        cc = hh * 4 + c
                        nc.tensor.matmul(pt[:, c * HID:(c + 1) * HID],
                                         lhsT=ht[:, cc * P:(cc + 1) * P],
                                         rhs=ident_r[:], is_transpose=True,
                                         start=True, stop=True)
                    sk3d = sk[:, i * NCH + hh * 4:i * NCH + hh * 4 + 4].rearrange(
                        "p (c o) -> p c o", o=1).to_broadcast([P, 4, HID])
                    nc.vector.tensor_tensor(
                        out=hp[:, hh * HALF:(hh + 1) * HALF].rearrange(
                            "p (c f) -> p c f", c=4),
                        in0=pt[:].bitcast(f32).rearrange("p (c f) -> p c f", c=4),
                        in1=sk3d, op=ALU.mult)
                hp32[g] = hp

            def phaseB2(g, l, i, maskadd):
                """Readouts + feature-major scaled h' for the next layer."""
                hp = hp32[g]
                kk = KKEEP[l]
                # masked tile for max readout (fp16, gpsimd)
                hm = hmpool.tile([P, HN], f16, tag="hm")
                ma3d = maskadd[:, i * NCH:(i + 1) * NCH].rearrange(
                    "p (c o) -> p c o", o=1).to_broadcast([P, NCH, HID])
                nc.gpsimd.tensor_tensor(
                    out=hm[:].rearrange("p (f c) -> p c f", c=NCH),
                    in0=hp[:].bitcast(f32).rearrange("p (c f) -> p c f", c=NCH),
                    in1=ma3d, op=ALU.add)
                # max readout (packed fp16 input -> DVE 2x mode)
                pmax = hmpool.tile([P, HID], f16, tag="pmax")
                nc.vector.tensor_reduce(
                    out=pmax[:], in_=hm[:].rearrange("p (f c) -> p f c", c=NCH),
                    axis=AX.X, op=ALU.max)
                ptm = psS.tile([P, HID], f16, tag="s")
                nc.tensor.matmul(ptm[:], lhsT=pmax[:], rhs=ident_h[:],
                                 is_transpose=True, start=True, stop=True)
                gmax = selpool.tile([P, 1], f16, tag="gmax")
                nc.vector.tensor_reduce(out=gmax[:], in_=ptm[:],
                                        axis=AX.X, op=ALU.max)
                nc.vector.tensor_tensor(out=zmax[:, g:g + 1],
                                        in0=zmax[:, g:g + 1], in1=gmax[:],
                                        op=ALU.add)
                # mean readout: column sums via ones-matmuls
                pm = psS.tile([HID, 1], f32, tag="s")
                for c in range(NCH):
                    nc.tensor.matmul(pm[:],
                                     lhsT=hp[:, c * HID:(c + 1) * HID].bitcast(f32),
                                     rhs=ones_f[:], start=(c == 0),
                                     stop=(c == NCH - 1), skip_group_check=True)
                nc.vector.scalar_tensor_tensor(
                    out=zmean[:, g:g + 1], in0=pm[:], scalar=1.0 / kk,
                    in1=zmean[:, g:g + 1], op0=ALU.mult, op1=ALU.add)
                # feature-major scaled h' for next layer's root term
                if l < 3:
                    hs = hspool.tile([HID, N], f32r, tag=f"hs{g % 4}",
                                     name=f"hs{g}_{l}")
                    for hh in range(2):
                        pts = psT.tile([P, HALF], f32r, tag="pt",
                                       name=f"pts{g}_{l}_{hh}")
                        for c in range(4):
                            cc = hh * 4 + c
                            nc.tensor.matmul(pts[:, c * P:(c + 1) * P],
                                             lhsT=hp[:, cc * HID:(cc + 1) * HID],
                                             rhs=ident_r[:], is_transpose=True,
                                             start=True, stop=True)
                        if hh == 0:
                            nc.scalar.copy(hs[:, 0:HALF], pts[:].bitcast(f32))
                        else:
                            nc.vector.tensor_copy(hs[:, HALF:N],
                                                  pts[:].bitcast(f32))
                    hT_s[g] = hs

            # ---------------- main loop: two pair-chains in flight ----------
            def pair_chain(pr):
                g0, g1 = 2 * pr, 2 * pr + 1
                keep = selpool.tile([P, 2 * NCH], f32, tag="keep", bufs=4,
                                    name=f"keep{pr}_0")
                nc.vector.memset(keep[:], 1.0)
                ucs = []
                for l in (1, 2, 3):
                    pz_pair = psS.tile([P, 2 * NCH], f32, tag="s",
                                       name=f"pz{pr}_{l}")
                    phaseA(g0, l, pz_pair, 0)
                    yield
                    phaseA(g1, l, pz_pair, 1)
                    yield
                    keep, sk, maskadd = selection(pr, l, pz_pair, keep, ucs)
                    yield
                    phaseB1(g0, l, 0, sk)
                    yield
                    phaseB1(g1, l, 1, sk)
                    yield
                    phaseB2(g0, l, 0, maskadd)
                    yield
                    phaseB2(g1, l, 1, maskadd)
                    yield

            def mlp(c0, c1):
                """3-layer MLP (fp32) over graph columns [c0, c1)."""
                w = c1 - c0
                sl = slice(c0, c1)
                pa1 = psS.tile([HID, w], f32, tag="s", name=f"pa1_{c0}")
                nc.tensor.matmul(pa1[:], lhsT=w_t["W_lin1a"][:],
                                 rhs=zmax[:, sl], start=True, stop=False,
                                 skip_group_check=True)
                nc.tensor.matmul(pa1[:], lhsT=w_t["W_lin1b"][:],
                                 rhs=zmean[:, sl], start=False, stop=True,
                                 skip_group_check=True)
                a1 = selpool.tile([HID, w], f32, tag="a1", name=f"a1_{c0}")
                nc.scalar.activation(a1[:], pa1[:], AF.Relu,
                                     bias=w_t["b_lin1"][:, 0:1])
                pa2 = psS.tile([64, w], f32, tag="s", name=f"pa2_{c0}")
                nc.tensor.matmul(pa2[:], lhsT=w_t["W_lin2"][:], rhs=a1[:],
                                 start=True, stop=True)
                a2 = selpool.tile([64, w], f32, tag="a2", name=f"a2_{c0}")
                nc.scalar.activation(a2[:], pa2[:], AF.Relu,
                                     bias=w_t["b_lin2"][:, 0:1])
                pa3 = psS.tile([1, w], f32, tag="s", name=f"pa3_{c0}")
                nc.tensor.matmul(pa3[:], lhsT=w_t["W_lin3"][:], rhs=a2[:],
                                 start=True, stop=True)
                a3 = selpool.tile([1, w], f32, tag="a3", name=f"a3_{c0}")
                nc.scalar.activation(a3[:], pa3[:], AF.Identity,
                                     bias=w_t["b_lin3"][:, 0:1])
                nc.sync.dma_start(out=d_out[:, sl], in_=a3[:])

            load_A(0)
            load_xT(0)
            load_xT(1)
            load_A(1)
            load_A(2)
            load_xT(2)
            load_xT(3)
            load_A(3)
            chains = [pair_chain(p) for p in range(G_PER_CORE // 2)]
            # start chain k+1 once chain k has advanced THRESH[k] yields
            THRESH = [11, 11, 6]
            progress = [0] * len(chains)
            done = [False] * len(chains)
            started = 1
            mlp_first_emitted = False
            while not all(done):
                for k in range(started):
                    if done[k]:
                        continue
                    try:
                        next(chains[k])
                        progress[k] += 1
                    except StopIteration:
                        done[k] = True
                        if done[0] and done[1] and not mlp_first_emitted:
                            mlp(0, 4)
                            mlp_first_emitted = True
                        if len(done) > 2 and done[2] and mlp_first_emitted \
                                and not getattr(mlp, "_mid", False):
                            mlp(4, 6)
                            mlp._mid = True
                    if (k == started - 1 and started < len(chains)
                            and progress[k] >= THRESH[k]):
                        for g in (2 * started + 2, 2 * started + 3):
                            if g < G_PER_CORE:
                                load_xT(g)
                                load_A(g)
                        started += 1
            mlp(6, G_PER_CORE)

            # (MLP emitted by the driver, split in two graph-halves)

    nc.compile()
    return nc


HALF512 = 512


def prepare_inputs(inputs):
    """Host index-preprocessing + sharding. Returns per-core input maps."""
    x = np.asarray(inputs["x"], np.float32)
    ei = np.asarray(inputs["edge_index"], np.int64)
    src = ei[0] % N
    dst = ei[1] % N
    gid = ei[0] // N

    maps = []
    for core in range(N_CORES):
        gs = range(core * G_PER_CORE, (core + 1) * G_PER_CORE)
        xs = np.empty((P, G_PER_CORE, NCH, IN_F), np.float32)
        xT = np.empty((G_PER_CORE, IN_F, N), np.float32)
        As = np.empty((G_PER_CORE, P, NCH * N), np.float32)
        for i, g in enumerate(gs):
            xg = x[g * N:(g + 1) * N]                       # [N, IN_F]
            xs[:, i] = xg.reshape(NCH, P, IN_F).transpose(1, 0, 2)
            xT[i] = xg.T
            m = gid == g
            A = np.zeros((N, N), np.float32)
            np.add.at(A, (src[m], dst[m]), 1.0)
            # device layout [p, (half, chunk, 512)]
            Ah = A.reshape(NCH, P, 2, HALF512).transpose(1, 2, 0, 3)
            As[i] = Ah.reshape(P, NCH * N)
        im = {"x_nm": xs.reshape(P, G_PER_CORE * NCH * IN_F), "A_sd": As,
              "xT": xT}
        for l in (1, 2, 3):
            im[f"W_rel{l}"] = np.asarray(inputs[f"W_rel{l}"], np.float32)
            im[f"W_root{l}"] = np.asarray(inputs[f"W_root{l}"], np.float32)
            im[f"b_rel{l}"] = np.asarray(inputs[f"b_rel{l}"], np.float32).reshape(HID, 1)
            im[f"w_pool{l}"] = np.asarray(inputs[f"w_pool{l}"], np.float32).reshape(HID, 1)
        W1 = np.asarray(inputs["W_lin1"], np.float32)
        im["W_lin1a"] = np.ascontiguousarray(W1[:HID])
        im["W_lin1b"] = np.ascontiguousarray(W1[HID:])
        im["b_lin1"] = np.asarray(inputs["b_lin1"], np.float32).reshape(HID, 1)
        im["W_lin2"] = np.asarray(inputs["W_lin2"], np.float32)
        im["b_lin2"] = np.asarray(inputs["b_lin2"], np.float32).reshape(64, 1)
        im["W_lin3"] = np.asarray(inputs["W_lin3"], np.float32)
        im["b_lin3"] = np.asarray(inputs["b_lin3"], np.float32).reshape(1, 1)
        maps.append(im)
    return maps


def run_on_device(inputs, trace=False):
    from concourse.bass_utils import run_bass_kernel_spmd
    nc = build_program()
    maps = prepare_inputs(inputs)
    res = run_bass_kernel_spmd(nc, maps, core_ids=list(range(N_CORES)),
                               trace=trace)
    outs = [res.results[c]["out"].reshape(-1) for c in range(N_CORES)]
    full = np.concatenate(outs).astype(np.float32).reshape(B_GRAPHS, 1)
    return full, res


def kernel(**inputs) -> np.ndarray:
    out, _ = run_on_device(inputs)
    return out


# revision 43
# speedup vs baseline: 1.0329x; 1.0329x over previous
"""Trainium2 Bass kernel for nn_Net_420906795534 (GNN: 3x GraphConv + TopKPooling + readout + MLP).

Sharding: data-parallel over graphs - 8 graphs per NeuronCore x 8 cores.
Host does index-only preprocessing: per-graph dense adjacency count matrices
(f32, exact) and layout reshapes. All float compute runs on device.

Device algorithm (v2, f32r pipeline):
  All heavy matmuls use float32r operands (FP22 reads, 1 cycle/row for
  moving dims >= 256) - no bf16 hi/lo split needed. Per layer:
    conv:    agg_T[f,d] = sum_c h_nm_c(f32r).T @ A_c(f32r)   (PE)
             h_T = relu(W_rel.T @ agg_T + W_root.T @ hTs + b) (PE f32r + ACT)
    pool:    u = (h.w)/||w||; exact jax.lax.top_k replication via the
             clip-at-XSAT lexicographic cascade (gpsimd kth_largest),
             batched across a PAIR of graphs ([128,16] tiles).
    readout: masked max in fp16 (DVE reduce + PE transpose), mean via
             ones-matmuls accumulated in PSUM.
  Two pair-chains of graphs are software-pipelined (generator interleave)
  so PE stays busy through the selection cascade; A tiles (f32) stream
  per graph with dst-half split DMAs.
"""
import sys
sys.path.insert(0, '/opt/trn_rl_repo')
import math
import numpy as np

B_GRAPHS, N, DEG = 64, 1024, 16
IN_F, HID = 20, 128
G_PER_CORE = 8
N_CORES = 8
P = 128
NCH = N // P  # 8 node chunks per graph
XSAT = np.float32(7.998811721801758)  # XLA-cpu f32 tanh saturation cutoff
K1, K2, K3 = 820, 656, 525           # ceil(0.8*n) chain
NDROP = {1: N - K1, 2: K1 - K2, 3: K2 - K3}      # 204, 164, 131
NVALID = {1: N, 2: K1, 3: K2}
KKEEP = {1: K1, 2: K2, 3: K3}
BIG = 1e20
INVALID = -1e30


def _quantile_for_rank(rank_m2: int, n_valid: int) -> float:
    """Return q so kth_largest's k_adj == rank_m2 exactly."""
    lo = int(math.ceil(rank_m2 * (1 << 32) / (n_valid - 1)))
    hi = int(math.ceil((rank_m2 + 1) * (1 << 32) / (n_valid - 1))) - 1
    omq = (lo + hi) // 2
    assert (omq * (n_valid - 1)) >> 32 == rank_m2
    return 1.0 - omq / (1 << 32)


def build_program():
    import concourse.bacc as bacc
    import concourse.mybir as mybir
    import concourse.tile as tile
    from concourse.masks import make_identity

    f32 = mybir.dt.float32
    f32r = mybir.dt.float32r
    f16 = mybir.dt.float16
    i32 = mybir.dt.int32
    AF = mybir.ActivationFunctionType
    ALU = mybir.AluOpType
    AX = mybir.AxisListType

    nc = bacc.Bacc("TRN2", target_bir_lowering=False, debug=False,
                   num_devices=N_CORES)

    # ---------------- DRAM I/O ----------------
    d_x = nc.dram_tensor("x_nm", [P, G_PER_CORE * NCH * IN_F], f32r,
                         kind="ExternalInput")
    d_xT = nc.dram_tensor("xT", [G_PER_CORE, IN_F, N], f32r,
                          kind="ExternalInput")
    d_A = nc.dram_tensor("A_sd", [G_PER_CORE, P, NCH * N], f32r,
                         kind="ExternalInput")
    d_w = {}
    for l, infl in ((1, IN_F), (2, HID), (3, HID)):
        d_w[f"W_rel{l}"] = nc.dram_tensor(f"W_rel{l}", [infl, HID], f32r,
                                          kind="ExternalInput")
        d_w[f"W_root{l}"] = nc.dram_tensor(f"W_root{l}", [infl, HID], f32r,
                                           kind="ExternalInput")
        d_w[f"b_rel{l}"] = nc.dram_tensor(f"b_rel{l}", [HID, 1], f32,
                                          kind="ExternalInput")
        d_w[f"w_pool{l}"] = nc.dram_tensor(f"w_pool{l}", [HID, 1], f32r,
                                           kind="ExternalInput")
    d_w["W_lin1a"] = nc.dram_tensor("W_lin1a", [HID, HID], f32, kind="ExternalInput")
    d_w["W_lin1b"] = nc.dram_tensor("W_lin1b", [HID, HID], f32, kind="ExternalInput")
    d_w["b_lin1"] = nc.dram_tensor("b_lin1", [HID, 1], f32, kind="ExternalInput")
    d_w["W_lin2"] = nc.dram_tensor("W_lin2", [HID, 64], f32, kind="ExternalInput")
    d_w["b_lin2"] = nc.dram_tensor("b_lin2", [64, 1], f32, kind="ExternalInput")
    d_w["W_lin3"] = nc.dram_tensor("W_lin3", [64, 1], f32, kind="ExternalInput")
    d_w["b_lin3"] = nc.dram_tensor("b_lin3", [1, 1], f32, kind="ExternalInput")
    d_out = nc.dram_tensor("out", [1, G_PER_CORE], f32, kind="ExternalOutput")

    HN = NCH * HID  # 1024
    HALF = 512

    with tile.TileContext(nc) as tc:
        with (
            tc.tile_pool(name="const", bufs=1) as cpool,
            tc.tile_pool(name="apool", bufs=4) as apool,
            tc.tile_pool(name="xtpool", bufs=2) as xtpool,
            tc.tile_pool(name="htpool", bufs=3) as htpool,
            tc.tile_pool(name="hppool", bufs=1) as hppool,
            tc.tile_pool(name="hspool", bufs=1) as hspool,
            tc.tile_pool(name="aggpool", bufs=2) as aggpool,
            tc.tile_pool(name="hmpool", bufs=2) as hmpool,
            tc.tile_pool(name="selpool", bufs=2) as selpool,
            tc.tile_pool(name="psA", bufs=2, space="PSUM") as psA,
            tc.tile_pool(name="psH", bufs=2, space="PSUM") as psH,
            tc.tile_pool(name="psT", bufs=2, space="PSUM") as psT,
            tc.tile_pool(name="psS", bufs=2, space="PSUM") as psS,
        ):
            # ---------- constants / weights ----------
            ident_f = cpool.tile([P, P], f32)
            make_identity(nc, ident_f[:])
            ident_r = cpool.tile([P, P], f32r)
            nc.scalar.copy(ident_r[:], ident_f[:])
            ident_h = cpool.tile([P, P], f16)
            nc.scalar.copy(ident_h[:], ident_f[:])
            ones_f = cpool.tile([P, 1], f32)
            nc.vector.memset(ones_f[:], 1.0)
            # node index (p + 128*c) replicated for both graphs of a pair
            idx1 = cpool.tile([P, NCH], f32)
            idx1_i = cpool.tile([P, NCH], i32)
            nc.gpsimd.iota(idx1_i[:], pattern=[[128, NCH]], base=0,
                           channel_multiplier=1)
            nc.vector.tensor_copy(idx1[:], idx1_i[:])
            idx_pair = cpool.tile([P, 2 * NCH], f32)
            for i in range(2):
                nc.vector.tensor_copy(idx_pair[:, i * NCH:(i + 1) * NCH], idx1[:])

            w_t = {}
            for name, dd in d_w.items():
                t = cpool.tile(list(dd.shape), dd.dtype, tag=name)
                nc.sync.dma_start(out=t[:], in_=dd[:])
                w_t[name] = t

            # x node-major (all graphs), f32r
            x_nm = cpool.tile([P, G_PER_CORE * NCH * IN_F], f32r)
            nc.sync.dma_start(out=x_nm[:], in_=d_x[:])

            # invnorm_l = 1/||w_pool_l|| replicated [P,1]
            invnorm = {}
            for l in (1, 2, 3):
                pnw = psS.tile([1, 1], f32, tag="s")
                nc.tensor.matmul(pnw[:], lhsT=w_t[f"w_pool{l}"][:].bitcast(f32),
                                 rhs=w_t[f"w_pool{l}"][:].bitcast(f32),
                                 start=True, stop=True)
                nrm = selpool.tile([1, 1], f32, tag="nrm")
                nc.scalar.activation(nrm[:], pnw[:], AF.Sqrt)
                inv = selpool.tile([1, 1], f32, tag="inv")
                nc.vector.reciprocal(inv[:], nrm[:])
                invr = cpool.tile([P, 1], f32, tag=f"invn{l}")
                nc.gpsimd.partition_broadcast(invr[:], inv[:], channels=P)
                invnorm[l] = invr

            # global readout accumulators [feat, graph]
            zmax = cpool.tile([P, G_PER_CORE], f32)
            zmean = cpool.tile([P, G_PER_CORE], f32)
            nc.vector.memset(zmax[:], 0.0)
            nc.vector.memset(zmean[:], 0.0)

            # ---------- A tile management (bufs=4 rotation) ----------
            A_t = {}
            xT_t = {}

            def load_xT(g):
                t = xtpool.tile([IN_F, N], f32r, tag="xT", name=f"xT{g}")
                nc.sync.dma_start(out=t[:], in_=d_xT[g])
                xT_t[g] = t

            def load_A(g):
                # A laid out [p, (half, chunk, 512)]: four contiguous-quarter
                # DMAs so agg matmuls start as soon as their slice lands
                t = apool.tile([P, NCH * N], f32r, tag="A", name=f"A{g}")
                Q = NCH * N // 4
                for q in range(4):
                    nc.sync.dma_start(out=t[:, q * Q:(q + 1) * Q],
                                      in_=d_A[g][:, q * Q:(q + 1) * Q])
                A_t[g] = t

            # per-graph state tiles (rotate via per-slot tags)
            hp32 = {}   # node-major scaled h' (f32r), agg lhsT of next layer
            hT_s = {}   # feature-major scaled h' (f32r), root rhs of next layer
            hT_new = {}  # feature-major unscaled h (f32r), transient per layer

            def phaseA(g, l, pz_pair, i):
                """conv + linear + relu + scores for graph g, layer l."""
                infl = IN_F if l == 1 else HID
                At = A_t[g]
                hTs_src = xT_t[g] if l == 1 else hT_s[g]
                aggT = aggpool.tile([infl, N], f32r, tag="aggT", name=f"aggT{g}_{l}")
                ht = htpool.tile([HID, N], f32r, tag="hT", name=f"hT{g}_{l}")
                for half in range(2):
                    sl = slice(half * HALF, (half + 1) * HALF)
                    pagg = psA.tile([infl, HALF], f32, tag="agg")
                    for c in range(NCH):
                        if l == 1:
                            lhs = x_nm[:, (g * NCH + c) * IN_F:(g * NCH + c + 1) * IN_F]
                        else:
                            lhs = hp32[g][:, c * HID:(c + 1) * HID]
                        nc.tensor.matmul(
                            pagg[:], lhsT=lhs,
                            rhs=At[:, (half * NCH + c) * HALF:(half * NCH + c + 1) * HALF],
                            start=(c == 0), stop=(c == NCH - 1),
                            skip_group_check=True)
                    nc.scalar.copy(aggT[:, sl], pagg[:])
                    ph = psH.tile([HID, HALF], f32, tag="ph")
                    nc.tensor.matmul(ph[:], lhsT=w_t[f"W_root{l}"][:],
                                     rhs=hTs_src[:, sl], start=True, stop=False,
                                     skip_group_check=True)
                    nc.tensor.matmul(ph[:], lhsT=w_t[f"W_rel{l}"][:],
                                     rhs=aggT[:, sl], start=False, stop=True,
                                     skip_group_check=True)
                    nc.scalar.activation(ht[:, sl], ph[:], AF.Relu,
                                         bias=w_t[f"b_rel{l}"][:, 0:1])
                hT_new[g] = ht
                # scores: pz[:, i*8+c] = h_chunk.T @ w_pool
                for c in range(NCH):
                    nc.tensor.matmul(
                        pz_pair[:, i * NCH + c:i * NCH + c + 1],
                        lhsT=ht[:, c * P:(c + 1) * P].bitcast(f32),
                        rhs=w_t[f"w_pool{l}"][:].bitcast(f32),
                        start=(c == 0), stop=(c == NCH - 1),
                        skip_group_check=True)

            def selection(pr, l, pz_pair, keep, ucs):
                """Batched pair top-k keep mask. Returns (keep_new, sk, maskadd)."""
                W = 2 * NCH
                nvalid, ndrop = NVALID[l], NDROP[l]
                u = selpool.tile([P, W], f32, tag="u")
                nc.scalar.activation(u[:], pz_pair[:], AF.Copy,
                                     scale=invnorm[l][:, 0:1])
                uc = selpool.tile([P, W], f32, tag=f"uc{l}")
                nc.vector.tensor_scalar(out=uc[:], in0=u[:], scalar1=float(XSAT),
                                        scalar2=float(-XSAT), op0=ALU.min,
                                        op1=ALU.max)
                ucs.append(uc)

                comps = [("u", t) for t in reversed(ucs)] + [("i", idx_pair)]
                bg = selpool.tile([P, W], f32, tag="bg")
                nc.vector.tensor_scalar(out=bg[:], in0=keep[:],
                                        scalar1=float(-INVALID),
                                        scalar2=float(INVALID),
                                        op0=ALU.mult, op1=ALU.add)
                ic = selpool.tile([P, W], f32, tag="ic")
                nc.vector.tensor_copy(ic[:], keep[:])
                dropped = selpool.tile([P, W], f32, tag="dropped")
                nc.vector.memset(dropped[:], 0.0)
                q = _quantile_for_rank(ndrop - 2, nvalid)
                for j, (kind, comp) in enumerate(comps):
                    key = selpool.tile([P, W], f32, tag="key")
                    nc.vector.tensor_tensor(out=key[:], in0=comp[:], in1=ic[:],
                                            op=ALU.mult)
                    if kind == "u":
                        nc.vector.scalar_tensor_tensor(
                            out=key[:], in0=key[:], scalar=-1.0, in1=bg[:],
                            op0=ALU.mult, op1=ALU.add)
                    else:
                        nc.vector.tensor_tensor(out=key[:], in0=key[:],
                                                in1=bg[:], op=ALU.add)
                    tv = selpool.tile([1, 4], f32, tag="tv")
                    for i in range(2):
                        nc.gpsimd.kth_largest(
                            tv[:, 2 * i:2 * i + 2],
                            key[:, i * NCH:(i + 1) * NCH],
                            n_per_lane=NCH, k=ndrop, quantile=q)
                    vrep = selpool.tile([P, 4], f32, tag="vrep")
                    nc.gpsimd.partition_broadcast(vrep[:], tv[:], channels=P)
                    v3d = vrep[:, 1::2].rearrange(
                        "p (g o) -> p g o", o=1).to_broadcast([P, 2, NCH])
                    last = (j == len(comps) - 1)
                    nd = selpool.tile([P, W], f32, tag="nd")
                    nc.vector.tensor_tensor(
                        out=nd[:].rearrange("p (g c) -> p g c", g=2),
                        in0=key[:].rearrange("p (g c) -> p g c", g=2),
                        in1=v3d, op=(ALU.is_ge if last else ALU.is_gt))
                    nc.vector.tensor_tensor(out=nd[:], in0=nd[:], in1=ic[:],
                                            op=ALU.mult)
                    nc.vector.tensor_tensor(out=dropped[:], in0=dropped[:],
                                            in1=nd[:], op=ALU.add)
                    if not last:
                        eq = selpool.tile([P, W], f32, tag="eq")
                        nc.vector.tensor_tensor(
                            out=eq[:].rearrange("p (g c) -> p g c", g=2),
                            in0=key[:].rearrange("p (g c) -> p g c", g=2),
                            in1=v3d, op=ALU.is_equal)
                        ic_new = selpool.tile([P, W], f32, tag="ic")
                        nc.vector.tensor_tensor(out=ic_new[:], in0=eq[:],
                                                in1=ic[:], op=ALU.mult)
                        safe = selpool.tile([P, W], f32, tag="safe")
                        nc.vector.tensor_tensor(out=safe[:], in0=ic[:],
                                                in1=ic_new[:], op=ALU.subtract)
                        nc.vector.tensor_tensor(out=safe[:], in0=safe[:],
                                                in1=nd[:], op=ALU.subtract)
                        nc.vector.scalar_tensor_tensor(
                            out=bg[:], in0=nd[:], scalar=float(BIG),
                            in1=bg[:], op0=ALU.mult, op1=ALU.add)
                        nc.vector.scalar_tensor_tensor(
                            out=bg[:], in0=safe[:], scalar=float(-BIG),
                            in1=bg[:], op0=ALU.mult, op1=ALU.add)
                        ic = ic_new
                keep_new = selpool.tile([P, W], f32, tag="keep", bufs=4,
                                        name=f"keep{pr}_{l}")
                nc.vector.tensor_tensor(out=keep_new[:], in0=keep[:],
                                        in1=dropped[:], op=ALU.subtract)
                s = selpool.tile([P, W], f32, tag="s")
                nc.scalar.activation(s[:], u[:], AF.Tanh)
                sk = selpool.tile([P, W], f32, tag="sk")
                nc.vector.tensor_tensor(out=sk[:], in0=s[:], in1=keep_new[:],
                                        op=ALU.mult)
                # mask offset finite in fp16 (-inf would trip finite checks)
                maskadd = selpool.tile([P, W], f32, tag="maskadd")
                nc.vector.tensor_scalar(out=maskadd[:], in0=keep_new[:],
                                        scalar1=60000.0, scalar2=-60000.0,
                                        op0=ALU.mult, op1=ALU.add)
                return keep_new, sk, maskadd

            def phaseB1(g, l, i, sk):
                """Transposes of h_T and scaled node-major h' (hp)."""
                ht = hT_new[g]
                hp = hppool.tile([P, HN], f32r, tag=f"hp{g % 4}",
                                 name=f"hp{g}_{l}")
                for hh in range(2):
                    pt = psT.tile([P, HALF], f32r, tag="pt", name=f"pt{g}_{l}_{hh}")
                    for c in range(4):
                        cc = hh * 4 + c
                        nc.tensor.matmul(pt[:, c * HID:(c + 1) * HID],
                                         lhsT=ht[:, cc * P:(cc + 1) * P],
                                         rhs=ident_r[:], is_transpose=True,
                                         start=True, stop=True)
                    sk3d = sk[:, i * NCH + hh * 4:i * NCH + hh * 4 + 4].rearrange(
                        "p (c o) -> p c o", o=1).to_broadcast([P, 4, HID])
                    nc.vector.tensor_tensor(
                        out=hp[:, hh * HALF:(hh + 1) * HALF].rearrange(
                            "p (c f) -> p c f", c=4),
                        in0=pt[:].bitcast(f32).rearrange("p (c f) -> p c f", c=4),
                        in1=sk3d, op=ALU.mult)
                hp32[g] = hp

            def phaseB2(g, l, i, maskadd):
                """Readouts + feature-major scaled h' for the next layer."""
                hp = hp32[g]
                kk = KKEEP[l]
                # masked tile for max readout (fp16, gpsimd)
                hm = hmpool.tile([P, HN], f16, tag="hm")
                ma3d = maskadd[:, i * NCH:(i + 1) * NCH].rearrange(
                    "p (c o) -> p c o", o=1).to_broadcast([P, NCH, HID])
                hmv = hm[:].rearrange("p (f c) -> p c f", c=NCH)
                hpv = hp[:].bitcast(f32).rearrange("p (c f) -> p c f", c=NCH)
                FH = HID // 2
                nc.gpsimd.tensor_tensor(
                    out=hmv[:, :, 0:FH], in0=hpv[:, :, 0:FH],
                    in1=ma3d[:, :, 0:FH], op=ALU.add)
                nc.vector.tensor_tensor(
                    out=hmv[:, :, FH:HID], in0=hpv[:, :, FH:HID],
                    in1=ma3d[:, :, FH:HID], op=ALU.add)
                # max readout (packed fp16 input -> DVE 2x mode)
                pmax = hmpool.tile([P, HID], f16, tag="pmax")
                nc.vector.tensor_reduce(
                    out=pmax[:], in_=hm[:].rearrange("p (f c) -> p f c", c=NCH),
                    axis=AX.X, op=ALU.max)
                ptm = psS.tile([P, HID], f16, tag="s")
                nc.tensor.matmul(ptm[:], lhsT=pmax[:], rhs=ident_h[:],
                                 is_transpose=True, start=True, stop=True)
                gmax = selpool.tile([P, 1], f16, tag="gmax")
                nc.vector.tensor_reduce(out=gmax[:], in_=ptm[:],
                                        axis=AX.X, op=ALU.max)
                nc.vector.tensor_tensor(out=zmax[:, g:g + 1],
                                        in0=zmax[:, g:g + 1], in1=gmax[:],
                                        op=ALU.add)
                # mean readout: column sums via ones-matmuls
                pm = psS.tile([HID, 1], f32, tag="s")
                for c in range(NCH):
                    nc.tensor.matmul(pm[:],
                                     lhsT=hp[:, c * HID:(c + 1) * HID].bitcast(f32),
                                     rhs=ones_f[:], start=(c == 0),
                                     stop=(c == NCH - 1), skip_group_check=True)
                nc.vector.scalar_tensor_tensor(
                    out=zmean[:, g:g + 1], in0=pm[:], scalar=1.0 / kk,
                    in1=zmean[:, g:g + 1], op0=ALU.mult, op1=ALU.add)
                # feature-major scaled h' for next layer's root term
                if l < 3:
                    hs = hspool.tile([HID, N], f32r, tag=f"hs{g % 4}",
                                     name=f"hs{g}_{l}")
                    for hh in range(2):
                        pts = psT.tile([P, HALF], f32r, tag="pt",
                                       name=f"pts{g}_{l}_{hh}")
                        for c in range(4):
                            cc = hh * 4 + c
                            nc.tensor.matmul(pts[:, c * P:(c + 1) * P],
                                             lhsT=hp[:, cc * HID:(cc + 1) * HID],
                                             rhs=ident_r[:], is_transpose=True,
                                             start=True, stop=True)
                        if hh == 0:
                            nc.scalar.copy(hs[:, 0:HALF], pts[:].bitcast(f32))
                        else:
                            nc.vector.tensor_copy(hs[:, HALF:N],
                                                  pts[:].bitcast(f32))
                    hT_s[g] = hs

            # ---------------- main loop: two pair-chains in flight ----------
            def pair_chain(pr):
                g0, g1 = 2 * pr, 2 * pr + 1
                keep = selpool.tile([P, 2 * NCH], f32, tag="keep", bufs=4,
                                    name=f"keep{pr}_0")
                nc.vector.memset(keep[:], 1.0)
                ucs = []
                for l in (1, 2, 3):
                    pz_pair = psS.tile([P, 2 * NCH], f32, tag="s",
                                       name=f"pz{pr}_{l}")
                    phaseA(g0, l, pz_pair, 0)
                    yield
                    phaseA(g1, l, pz_pair, 1)
                    yield
                    keep, sk, maskadd = selection(pr, l, pz_pair, keep, ucs)
                    yield
                    phaseB1(g0, l, 0, sk)
                    yield
                    phaseB1(g1, l, 1, sk)
                    yield
                    phaseB2(g0, l, 0, maskadd)
                    yield
                    phaseB2(g1, l, 1, maskadd)
                    yield

            def mlp(c0, c1):
                """3-layer MLP (fp32) over graph columns [c0, c1)."""
                w = c1 - c0
                sl = slice(c0, c1)
                pa1 = psS.tile([HID, w], f32, tag="s", name=f"pa1_{c0}")
                nc.tensor.matmul(pa1[:], lhsT=w_t["W_lin1a"][:],
                                 rhs=zmax[:, sl], start=True, stop=False,
                                 skip_group_check=True)
                nc.tensor.matmul(pa1[:], lhsT=w_t["W_lin1b"][:],
                                 rhs=zmean[:, sl], start=False, stop=True,
                                 skip_group_check=True)
                a1 = selpool.tile([HID, w], f32, tag="a1", name=f"a1_{c0}")
                nc.scalar.activation(a1[:], pa1[:], AF.Relu,
                                     bias=w_t["b_lin1"][:, 0:1])
                pa2 = psS.tile([64, w], f32, tag="s", name=f"pa2_{c0}")
                nc.tensor.matmul(pa2[:], lhsT=w_t["W_lin2"][:], rhs=a1[:],
                                 start=True, stop=True)
                a2 = selpool.tile([64, w], f32, tag="a2", name=f"a2_{c0}")
                nc.scalar.activation(a2[:], pa2[:], AF.Relu,
                                     bias=w_t["b_lin2"][:, 0:1])
                pa3 = psS.tile([1, w], f32, tag="s", name=f"pa3_{c0}")
                nc.tensor.matmul(pa3[:], lhsT=w_t["W_lin3"][:], rhs=a2[:],
                                 start=True, stop=True)
                a3 = selpool.tile([1, w], f32, tag="a3", name=f"a3_{c0}")
                nc.scalar.activation(a3[:], pa3[:], AF.Identity,
                                     bias=w_t["b_lin3"][:, 0:1])
                nc.sync.dma_start(out=d_out[:, sl], in_=a3[:])

            load_A(0)
            load_xT(0)
            load_xT(1)
            load_A(1)
            load_A(2)
            load_xT(2)
            load_xT(3)
            load_A(3)
            chains = [pair_chain(p) for p in range(G_PER_CORE // 2)]
            # start chain k+1 once chain k has advanced THRESH[k] yields
            THRESH = [11, 11, 6]
            progress = [0] * len(chains)
            done = [False] * len(chains)
            started = 1
            mlp_first_emitted = False
            while not all(done):
                for k in range(started):
                    if done[k]:
                        continue
                    try:
                        next(chains[k])
                        progress[k] += 1
                    except StopIteration:
                        done[k] = True
                        if done[0] and done[1] and not mlp_first_emitted:
                            mlp(0, 4)
                            mlp_first_emitted = True
                        if len(done) > 2 and done[2] and mlp_first_emitted \
                                and not getattr(mlp, "_mid", False):
                            mlp(4, 6)
                            mlp._mid = True
                    if (k == started - 1 and started < len(chains)
                            and progress[k] >= THRESH[k]):
                        for g in (2 * started + 2, 2 * started + 3):
                            if g < G_PER_CORE:
                                load_xT(g)
                                load_A(g)
                        started += 1
            mlp(6, G_PER_CORE)

            # (MLP emitted by the driver, split in two graph-halves)

    nc.compile()
    return nc


HALF512 = 512


def prepare_inputs(inputs):
    """Host index-preprocessing + sharding. Returns per-core input maps."""
    x = np.asarray(inputs["x"], np.float32)
    ei = np.asarray(inputs["edge_index"], np.int64)
    src = ei[0] % N
    dst = ei[1] % N
    gid = ei[0] // N

    maps = []
    for core in range(N_CORES):
        gs = range(core * G_PER_CORE, (core + 1) * G_PER_CORE)
        xs = np.empty((P, G_PER_CORE, NCH, IN_F), np.float32)
        xT = np.empty((G_PER_CORE, IN_F, N), np.float32)
        As = np.empty((G_PER_CORE, P, NCH * N), np.float32)
        for i, g in enumerate(gs):
            xg = x[g * N:(g + 1) * N]                       # [N, IN_F]
            xs[:, i] = xg.reshape(NCH, P, IN_F).transpose(1, 0, 2)
            xT[i] = xg.T
            m = gid == g
            A = np.zeros((N, N), np.float32)
            np.add.at(A, (src[m], dst[m]), 1.0)
            # device layout [p, (half, chunk, 512)]
            Ah = A.reshape(NCH, P, 2, HALF512).transpose(1, 2, 0, 3)
            As[i] = Ah.reshape(P, NCH * N)
        im = {"x_nm": xs.reshape(P, G_PER_CORE * NCH * IN_F), "A_sd": As,
              "xT": xT}
        for l in (1, 2, 3):
            im[f"W_rel{l}"] = np.asarray(inputs[f"W_rel{l}"], np.float32)
            im[f"W_root{l}"] = np.asarray(inputs[f"W_root{l}"], np.float32)
            im[f"b_rel{l}"] = np.asarray(inputs[f"b_rel{l}"], np.float32).reshape(HID, 1)
            im[f"w_pool{l}"] = np.asarray(inputs[f"w_pool{l}"], np.float32).reshape(HID, 1)
        W1 = np.asarray(inputs["W_lin1"], np.float32)
        im["W_lin1a"] = np.ascontiguousarray(W1[:HID])
        im["W_lin1b"] = np.ascontiguousarray(W1[HID:])
        im["b_lin1"] = np.asarray(inputs["b_lin1"], np.float32).reshape(HID, 1)
        im["W_lin2"] = np.asarray(inputs["W_lin2"], np.float32)
        im["b_lin2"] = np.asarray(inputs["b_lin2"], np.float32).reshape(64, 1)
        im["W_lin3"] = np.asarray(inputs["W_lin3"], np.float32)
        im["b_lin3"] = np.asarray(inputs["b_lin3"], np.float32).reshape(1, 1)
        maps.append(im)
    return maps


def run_on_device(inputs, trace=False):
    from concourse.bass_utils import run_bass_kernel_spmd
    nc = build_program()
    maps = prepare_inputs(inputs)
    res = run_bass_kernel_spmd(nc, maps, core_ids=list(range(N_CORES)),
                               trace=trace)
    outs = [res.results[c]["out"].reshape(-1) for c in range(N_CORES)]
    full = np.concatenate(outs).astype(np.float32).reshape(B_GRAPHS, 1)
    return full, res


def kernel(**inputs) -> np.ndarray:
    out, _ = run_on_device(inputs)
    return out


# revision 48
# speedup vs baseline: 1.0462x; 1.0129x over previous
"""Trainium2 Bass kernel for nn_Net_420906795534 (GNN: 3x GraphConv + TopKPooling + readout + MLP).

Sharding: data-parallel over graphs - 8 graphs per NeuronCore x 8 cores.
Host does index-only preprocessing: per-graph dense adjacency count matrices
(f32, exact) and layout reshapes. All float compute runs on device.

Device algorithm (v2, f32r pipeline):
  All heavy matmuls use float32r operands (FP22 reads, 1 cycle/row for
  moving dims >= 256) - no bf16 hi/lo split needed. Per layer:
    conv:    agg_T[f,d] = sum_c h_nm_c(f32r).T @ A_c(f32r)   (PE)
             h_T = relu(W_rel.T @ agg_T + W_root.T @ hTs + b) (PE f32r + ACT)
    pool:    u = (h.w)/||w||; exact jax.lax.top_k replication via the
             clip-at-XSAT lexicographic cascade (gpsimd kth_largest),
             batched across a PAIR of graphs ([128,16] tiles).
    readout: masked max in fp16 (DVE reduce + PE transpose), mean via
             ones-matmuls accumulated in PSUM.
  Two pair-chains of graphs are software-pipelined (generator interleave)
  so PE stays busy through the selection cascade; A tiles (f32) stream
  per graph with dst-half split DMAs.
"""
import sys
sys.path.insert(0, '/opt/trn_rl_repo')
import math
import numpy as np

B_GRAPHS, N, DEG = 64, 1024, 16
IN_F, HID = 20, 128
G_PER_CORE = 8
N_CORES = 8
P = 128
NCH = N // P  # 8 node chunks per graph
XSAT = np.float32(7.998811721801758)  # XLA-cpu f32 tanh saturation cutoff
K1, K2, K3 = 820, 656, 525           # ceil(0.8*n) chain
NDROP = {1: N - K1, 2: K1 - K2, 3: K2 - K3}      # 204, 164, 131
NVALID = {1: N, 2: K1, 3: K2}
KKEEP = {1: K1, 2: K2, 3: K3}
BIG = 1e20
INVALID = -1e30


def _quantile_for_rank(rank_m2: int, n_valid: int) -> float:
    """Return q so kth_largest's k_adj == rank_m2 exactly."""
    lo = int(math.ceil(rank_m2 * (1 << 32) / (n_valid - 1)))
    hi = int(math.ceil((rank_m2 + 1) * (1 << 32) / (n_valid - 1))) - 1
    omq = (lo + hi) // 2
    assert (omq * (n_valid - 1)) >> 32 == rank_m2
    return 1.0 - omq / (1 << 32)


def build_program():
    import concourse.bacc as bacc
    import concourse.mybir as mybir
    import concourse.tile as tile
    from concourse.masks import make_identity

    f32 = mybir.dt.float32
    f32r = mybir.dt.float32r
    f16 = mybir.dt.float16
    i32 = mybir.dt.int32
    AF = mybir.ActivationFunctionType
    ALU = mybir.AluOpType
    AX = mybir.AxisListType

    nc = bacc.Bacc("TRN2", target_bir_lowering=False, debug=False,
                   num_devices=N_CORES)

    # ---------------- DRAM I/O ----------------
    d_x = nc.dram_tensor("x_nm", [P, G_PER_CORE * NCH * IN_F], f32r,
                         kind="ExternalInput")
    d_xT = nc.dram_tensor("xT", [G_PER_CORE, IN_F, N], f32r,
                          kind="ExternalInput")
    d_A = nc.dram_tensor("A_sd", [G_PER_CORE, P, NCH * N], f32r,
                         kind="ExternalInput")
    d_w = {}
    for l, infl in ((1, IN_F), (2, HID), (3, HID)):
        d_w[f"W_rel{l}"] = nc.dram_tensor(f"W_rel{l}", [infl, HID], f32r,
                                          kind="ExternalInput")
        d_w[f"W_root{l}"] = nc.dram_tensor(f"W_root{l}", [infl, HID], f32r,
                                           kind="ExternalInput")
        d_w[f"b_rel{l}"] = nc.dram_tensor(f"b_rel{l}", [HID, 1], f32,
                                          kind="ExternalInput")
        d_w[f"w_pool{l}"] = nc.dram_tensor(f"w_pool{l}", [HID, 1], f32r,
                                           kind="ExternalInput")
    d_w["W_lin1a"] = nc.dram_tensor("W_lin1a", [HID, HID], f32, kind="ExternalInput")
    d_w["W_lin1b"] = nc.dram_tensor("W_lin1b", [HID, HID], f32, kind="ExternalInput")
    d_w["b_lin1"] = nc.dram_tensor("b_lin1", [HID, 1], f32, kind="ExternalInput")
    d_w["W_lin2"] = nc.dram_tensor("W_lin2", [HID, 64], f32, kind="ExternalInput")
    d_w["b_lin2"] = nc.dram_tensor("b_lin2", [64, 1], f32, kind="ExternalInput")
    d_w["W_lin3"] = nc.dram_tensor("W_lin3", [64, 1], f32, kind="ExternalInput")
    d_w["b_lin3"] = nc.dram_tensor("b_lin3", [1, 1], f32, kind="ExternalInput")
    d_out = nc.dram_tensor("out", [1, G_PER_CORE], f32, kind="ExternalOutput")

    HN = NCH * HID  # 1024
    HALF = 512

    with tile.TileContext(nc) as tc:
        with (
            tc.tile_pool(name="const", bufs=1) as cpool,
            tc.tile_pool(name="apool", bufs=4) as apool,
            tc.tile_pool(name="xtpool", bufs=2) as xtpool,
            tc.tile_pool(name="htpool", bufs=3) as htpool,
            tc.tile_pool(name="hppool", bufs=1) as hppool,
            tc.tile_pool(name="hspool", bufs=1) as hspool,
            tc.tile_pool(name="aggpool", bufs=2) as aggpool,
            tc.tile_pool(name="hmpool", bufs=2) as hmpool,
            tc.tile_pool(name="selpool", bufs=2) as selpool,
            tc.tile_pool(name="psA", bufs=2, space="PSUM") as psA,
            tc.tile_pool(name="psH", bufs=2, space="PSUM") as psH,
            tc.tile_pool(name="psT", bufs=2, space="PSUM") as psT,
            tc.tile_pool(name="psS", bufs=2, space="PSUM") as psS,
        ):
            # ---------- constants / weights ----------
            ident_f = cpool.tile([P, P], f32)
            make_identity(nc, ident_f[:])
            ident_r = cpool.tile([P, P], f32r)
            nc.scalar.copy(ident_r[:], ident_f[:])
            ident_h = cpool.tile([P, P], f16)
            nc.scalar.copy(ident_h[:], ident_f[:])
            ones_f = cpool.tile([P, 1], f32)
            nc.vector.memset(ones_f[:], 1.0)
            # node index (p + 128*c) replicated for both graphs of a pair
            idx1 = cpool.tile([P, NCH], f32)
            idx1_i = cpool.tile([P, NCH], i32)
            nc.gpsimd.iota(idx1_i[:], pattern=[[128, NCH]], base=0,
                           channel_multiplier=1)
            nc.vector.tensor_copy(idx1[:], idx1_i[:])
            idx_pair = cpool.tile([P, 2 * NCH], f32)
            for i in range(2):
                nc.vector.tensor_copy(idx_pair[:, i * NCH:(i + 1) * NCH], idx1[:])

            w_t = {}
            for name, dd in d_w.items():
                t = cpool.tile(list(dd.shape), dd.dtype, tag=name)
                nc.sync.dma_start(out=t[:], in_=dd[:])
                w_t[name] = t

            # x node-major (all graphs), f32r
            x_nm = cpool.tile([P, G_PER_CORE * NCH * IN_F], f32r)
            nc.sync.dma_start(out=x_nm[:], in_=d_x[:])

            # invnorm_l = 1/||w_pool_l|| replicated [P,1]
            invnorm = {}
            for l in (1, 2, 3):
                pnw = psS.tile([1, 1], f32, tag="s")
                nc.tensor.matmul(pnw[:], lhsT=w_t[f"w_pool{l}"][:].bitcast(f32),
                                 rhs=w_t[f"w_pool{l}"][:].bitcast(f32),
                                 start=True, stop=True)
                nrm = selpool.tile([1, 1], f32, tag="nrm")
                nc.scalar.activation(nrm[:], pnw[:], AF.Sqrt)
                inv = selpool.tile([1, 1], f32, tag="inv")
                nc.vector.reciprocal(inv[:], nrm[:])
                invr = cpool.tile([P, 1], f32, tag=f"invn{l}")
                nc.gpsimd.partition_broadcast(invr[:], inv[:], channels=P)
                invnorm[l] = invr

            # global readout accumulators [feat, graph]
            zmax = cpool.tile([P, G_PER_CORE], f32)
            zmean = cpool.tile([P, G_PER_CORE], f32)
            nc.vector.memset(zmax[:], 0.0)
            nc.vector.memset(zmean[:], 0.0)

            # ---------- A tile management (bufs=4 rotation) ----------
            A_t = {}
            xT_t = {}

            def load_xT(g):
                t = xtpool.tile([IN_F, N], f32r, tag="xT", name=f"xT{g}")
                nc.sync.dma_start(out=t[:], in_=d_xT[g])
                xT_t[g] = t

            def load_A(g):
                # A laid out [p, (half, chunk, 512)]: four contiguous-quarter
                # DMAs so agg matmuls start as soon as their slice lands
                t = apool.tile([P, NCH * N], f32r, tag="A", name=f"A{g}")
                Q = NCH * N // 4
                for q in range(4):
                    nc.sync.dma_start(out=t[:, q * Q:(q + 1) * Q],
                                      in_=d_A[g][:, q * Q:(q + 1) * Q])
                A_t[g] = t

            # per-graph state tiles (rotate via per-slot tags)
            hp32 = {}   # node-major scaled h' (f32r), agg lhsT of next layer
            hT_s = {}   # feature-major scaled h' (f32r), root rhs of next layer
            hT_new = {}  # feature-major unscaled h (f32r), transient per layer

            def phaseA(g, l, pz_pair, i):
                """conv + linear + relu + scores for graph g, layer l."""
                infl = IN_F if l == 1 else HID
                At = A_t[g]
                hTs_src = xT_t[g] if l == 1 else hT_s[g]
                aggT = aggpool.tile([infl, N], f32r, tag="aggT", name=f"aggT{g}_{l}")
                ht = htpool.tile([HID, N], f32r, tag="hT", name=f"hT{g}_{l}")
                for half in range(2):
                    sl = slice(half * HALF, (half + 1) * HALF)
                    pagg = psA.tile([infl, HALF], f32, tag="agg")
                    for c in range(NCH):
                        if l == 1:
                            lhs = x_nm[:, (g * NCH + c) * IN_F:(g * NCH + c + 1) * IN_F]
                        else:
                            lhs = hp32[g][:, c * HID:(c + 1) * HID]
                        nc.tensor.matmul(
                            pagg[:], lhsT=lhs,
                            rhs=At[:, (half * NCH + c) * HALF:(half * NCH + c + 1) * HALF],
                            start=(c == 0), stop=(c == NCH - 1),
                            skip_group_check=True)
                    nc.scalar.copy(aggT[:, sl], pagg[:])
                    ph = psH.tile([HID, HALF], f32, tag="ph")
                    nc.tensor.matmul(ph[:], lhsT=w_t[f"W_root{l}"][:],
                                     rhs=hTs_src[:, sl], start=True, stop=False,
                                     skip_group_check=True)
                    nc.tensor.matmul(ph[:], lhsT=w_t[f"W_rel{l}"][:],
                                     rhs=aggT[:, sl], start=False, stop=True,
                                     skip_group_check=True)
                    nc.scalar.activation(ht[:, sl], ph[:], AF.Relu,
                                         bias=w_t[f"b_rel{l}"][:, 0:1])
                hT_new[g] = ht
                # scores: pz[:, i*8+c] = h_chunk.T @ w_pool
                for c in range(NCH):
                    nc.tensor.matmul(
                        pz_pair[:, i * NCH + c:i * NCH + c + 1],
                        lhsT=ht[:, c * P:(c + 1) * P].bitcast(f32),
                        rhs=w_t[f"w_pool{l}"][:].bitcast(f32),
                        start=(c == 0), stop=(c == NCH - 1),
                        skip_group_check=True)

            def selection(pr, l, pz_pair, keep, ucs):
                """Batched pair top-k keep mask. Returns (keep_new, sk, maskadd)."""
                W = 2 * NCH
                nvalid, ndrop = NVALID[l], NDROP[l]
                u = selpool.tile([P, W], f32, tag="u")
                nc.scalar.activation(u[:], pz_pair[:], AF.Copy,
                                     scale=invnorm[l][:, 0:1])
                uc = selpool.tile([P, W], f32, tag=f"uc{l}")
                nc.vector.tensor_scalar(out=uc[:], in0=u[:], scalar1=float(XSAT),
                                        scalar2=float(-XSAT), op0=ALU.min,
                                        op1=ALU.max)
                ucs.append(uc)

                comps = [("u", t) for t in reversed(ucs)] + [("i", idx_pair)]
                bg = selpool.tile([P, W], f32, tag="bg")
                nc.vector.tensor_scalar(out=bg[:], in0=keep[:],
                                        scalar1=float(-INVALID),
                                        scalar2=float(INVALID),
                                        op0=ALU.mult, op1=ALU.add)
                ic = selpool.tile([P, W], f32, tag="ic")
                nc.vector.tensor_copy(ic[:], keep[:])
                dropped = selpool.tile([P, W], f32, tag="dropped")
                nc.vector.memset(dropped[:], 0.0)
                q = _quantile_for_rank(ndrop - 2, nvalid)
                for j, (kind, comp) in enumerate(comps):
                    key = selpool.tile([P, W], f32, tag="key")
                    nc.vector.tensor_tensor(out=key[:], in0=comp[:], in1=ic[:],
                                            op=ALU.mult)
                    if kind == "u":
                        nc.vector.scalar_tensor_tensor(
                            out=key[:], in0=key[:], scalar=-1.0, in1=bg[:],
                            op0=ALU.mult, op1=ALU.add)
                    else:
                        nc.vector.tensor_tensor(out=key[:], in0=key[:],
                                                in1=bg[:], op=ALU.add)
                    tv = selpool.tile([1, 4], f32, tag="tv")
                    for i in range(2):
                        nc.gpsimd.kth_largest(
                            tv[:, 2 * i:2 * i + 2],
                            key[:, i * NCH:(i + 1) * NCH],
                            n_per_lane=NCH, k=ndrop, quantile=q)
                    vrep = selpool.tile([P, 4], f32, tag="vrep")
                    nc.gpsimd.partition_broadcast(vrep[:], tv[:], channels=P)
                    v3d = vrep[:, 1::2].rearrange(
                        "p (g o) -> p g o", o=1).to_broadcast([P, 2, NCH])
                    last = (j == len(comps) - 1)
                    nd = selpool.tile([P, W], f32, tag="nd")
                    nc.vector.tensor_tensor(
                        out=nd[:].rearrange("p (g c) -> p g c", g=2),
                        in0=key[:].rearrange("p (g c) -> p g c", g=2),
                        in1=v3d, op=(ALU.is_ge if last else ALU.is_gt))
                    nc.vector.tensor_tensor(out=nd[:], in0=nd[:], in1=ic[:],
                                            op=ALU.mult)
                    nc.vector.tensor_tensor(out=dropped[:], in0=dropped[:],
                                            in1=nd[:], op=ALU.add)
                    if not last:
                        eq = selpool.tile([P, W], f32, tag="eq")
                        nc.vector.tensor_tensor(
                            out=eq[:].rearrange("p (g c) -> p g c", g=2),
                            in0=key[:].rearrange("p (g c) -> p g c", g=2),
                            in1=v3d, op=ALU.is_equal)
                        ic_new = selpool.tile([P, W], f32, tag="ic")
                        nc.vector.tensor_tensor(out=ic_new[:], in0=eq[:],
                                                in1=ic[:], op=ALU.mult)
                        safe = selpool.tile([P, W], f32, tag="safe")
                        nc.vector.tensor_tensor(out=safe[:], in0=ic[:],
                                                in1=ic_new[:], op=ALU.subtract)
                        nc.vector.tensor_tensor(out=safe[:], in0=safe[:],
                                                in1=nd[:], op=ALU.subtract)
                        nc.vector.scalar_tensor_tensor(
                            out=bg[:], in0=nd[:], scalar=float(BIG),
                            in1=bg[:], op0=ALU.mult, op1=ALU.add)
                        nc.vector.scalar_tensor_tensor(
                            out=bg[:], in0=safe[:], scalar=float(-BIG),
                            in1=bg[:], op0=ALU.mult, op1=ALU.add)
                        ic = ic_new
                keep_new = selpool.tile([P, W], f32, tag="keep", bufs=4,
                                        name=f"keep{pr}_{l}")
                nc.vector.tensor_tensor(out=keep_new[:], in0=keep[:],
                                        in1=dropped[:], op=ALU.subtract)
                s = selpool.tile([P, W], f32, tag="s")
                nc.scalar.activation(s[:], u[:], AF.Tanh)
                sk = selpool.tile([P, W], f32, tag="sk")
                nc.vector.tensor_tensor(out=sk[:], in0=s[:], in1=keep_new[:],
                                        op=ALU.mult)
                # mask offset finite in fp16 (-inf would trip finite checks)
                maskadd = selpool.tile([P, W], f32, tag="maskadd")
                nc.vector.tensor_scalar(out=maskadd[:], in0=keep_new[:],
                                        scalar1=60000.0, scalar2=-60000.0,
                                        op0=ALU.mult, op1=ALU.add)
                return keep_new, sk, maskadd

            def phaseB1(g, l, i, sk):
                """Transposes of h_T and scaled node-major h' (hp)."""
                ht = hT_new[g]
                hp = hppool.tile([P, HN], f32r, tag=f"hp{g % 4}",
                                 name=f"hp{g}_{l}")
                for hh in range(2):
                    pt = psT.tile([P, HALF], f32r, tag="pt", name=f"pt{g}_{l}_{hh}")
                    for c in range(4):
                        cc = hh * 4 + c
                        nc.tensor.matmul(pt[:, c * HID:(c + 1) * HID],
                                         lhsT=ht[:, cc * P:(cc + 1) * P],
                                         rhs=ident_r[:], is_transpose=True,
                                         start=True, stop=True)
                    sk3d = sk[:, i * NCH + hh * 4:i * NCH + hh * 4 + 4].rearrange(
                        "p (c o) -> p c o", o=1).to_broadcast([P, 4, HID])
                    nc.vector.tensor_tensor(
                        out=hp[:, hh * HALF:(hh + 1) * HALF].rearrange(
                            "p (c f) -> p c f", c=4),
                        in0=pt[:].bitcast(f32).rearrange("p (c f) -> p c f", c=4),
                        in1=sk3d, op=ALU.mult)
                hp32[g] = hp

            def phaseB2(g, l, i, maskadd):
                """Readouts + feature-major scaled h' for the next layer."""
                hp = hp32[g]
                kk = KKEEP[l]
                # masked tile for max readout (fp16, gpsimd)
                hm = hmpool.tile([P, HN], f16, tag="hm")
                ma3d = maskadd[:, i * NCH:(i + 1) * NCH].rearrange(
                    "p (c o) -> p c o", o=1).to_broadcast([P, NCH, HID])
                hmv = hm[:].rearrange("p (f c) -> p c f", c=NCH)
                hpv = hp[:].bitcast(f32).rearrange("p (c f) -> p c f", c=NCH)
                FH = HID // 2
                nc.gpsimd.tensor_tensor(
                    out=hmv[:, :, 0:FH], in0=hpv[:, :, 0:FH],
                    in1=ma3d[:, :, 0:FH], op=ALU.add)
                nc.vector.tensor_tensor(
                    out=hmv[:, :, FH:HID], in0=hpv[:, :, FH:HID],
                    in1=ma3d[:, :, FH:HID], op=ALU.add)
                # max readout (packed fp16 input -> DVE 2x mode)
                pmax = hmpool.tile([P, HID], f16, tag="pmax")
                nc.vector.tensor_reduce(
                    out=pmax[:], in_=hm[:].rearrange("p (f c) -> p f c", c=NCH),
                    axis=AX.X, op=ALU.max)
                ptm = psS.tile([P, HID], f16, tag="s")
                nc.tensor.matmul(ptm[:], lhsT=pmax[:], rhs=ident_h[:],
                                 is_transpose=True, start=True, stop=True)
                gmax = selpool.tile([P, 1], f16, tag="gmax")
                nc.vector.tensor_reduce(out=gmax[:], in_=ptm[:],
                                        axis=AX.X, op=ALU.max)
                nc.vector.tensor_tensor(out=zmax[:, g:g + 1],
                                        in0=zmax[:, g:g + 1], in1=gmax[:],
                                        op=ALU.add)
                # mean readout: column sums via ones-matmuls
                pm = psS.tile([HID, 1], f32, tag="s")
                for c in range(NCH):
                    nc.tensor.matmul(pm[:],
                                     lhsT=hp[:, c * HID:(c + 1) * HID].bitcast(f32),
                                     rhs=ones_f[:], start=(c == 0),
                                     stop=(c == NCH - 1), skip_group_check=True)
                nc.vector.scalar_tensor_tensor(
                    out=zmean[:, g:g + 1], in0=pm[:], scalar=1.0 / kk,
                    in1=zmean[:, g:g + 1], op0=ALU.mult, op1=ALU.add)
                # feature-major scaled h' for next layer's root term
                if l < 3:
                    hs = hspool.tile([HID, N], f32r, tag=f"hs{g % 4}",
                                     name=f"hs{g}_{l}")
                    for hh in range(2):
                        pts = psT.tile([P, HALF], f32r, tag="pt",
                                       name=f"pts{g}_{l}_{hh}")
                        for c in range(4):
                            cc = hh * 4 + c
                            nc.tensor.matmul(pts[:, c * P:(c + 1) * P],
                                             lhsT=hp[:, cc * HID:(cc + 1) * HID],
                                             rhs=ident_r[:], is_transpose=True,
                                             start=True, stop=True)
                        if hh == 0:
                            nc.scalar.copy(hs[:, 0:HALF], pts[:].bitcast(f32))
                        else:
                            nc.vector.tensor_copy(hs[:, HALF:N],
                                                  pts[:].bitcast(f32))
                    hT_s[g] = hs

            # ---------------- main loop: two pair-chains in flight ----------
            def pair_chain(pr):
                g0, g1 = 2 * pr, 2 * pr + 1
                keep = selpool.tile([P, 2 * NCH], f32, tag="keep", bufs=4,
                                    name=f"keep{pr}_0")
                nc.vector.memset(keep[:], 1.0)
                ucs = []
                for l in (1, 2, 3):
                    pz_pair = psS.tile([P, 2 * NCH], f32, tag="s",
                                       name=f"pz{pr}_{l}")
                    phaseA(g0, l, pz_pair, 0)
                    yield
                    phaseA(g1, l, pz_pair, 1)
                    yield
                    keep, sk, maskadd = selection(pr, l, pz_pair, keep, ucs)
                    yield
                    phaseB1(g0, l, 0, sk)
                    yield
                    phaseB1(g1, l, 1, sk)
                    yield
                    phaseB2(g0, l, 0, maskadd)
                    yield
                    phaseB2(g1, l, 1, maskadd)
                    yield

            def mlp(c0, c1):
                """3-layer MLP (fp32) over graph columns [c0, c1)."""
                w = c1 - c0
                sl = slice(c0, c1)
                pa1 = psS.tile([HID, w], f32, tag="s", name=f"pa1_{c0}")
                nc.tensor.matmul(pa1[:], lhsT=w_t["W_lin1a"][:],
                                 rhs=zmax[:, sl], start=True, stop=False,
                                 skip_group_check=True)
                nc.tensor.matmul(pa1[:], lhsT=w_t["W_lin1b"][:],
                                 rhs=zmean[:, sl], start=False, stop=True,
                                 skip_group_check=True)
                a1 = selpool.tile([HID, w], f32, tag="a1", name=f"a1_{c0}")
                nc.scalar.activation(a1[:], pa1[:], AF.Relu,
                                     bias=w_t["b_lin1"][:, 0:1])
                pa2 = psS.tile([64, w], f32, tag="s", name=f"pa2_{c0}")
                nc.tensor.matmul(pa2[:], lhsT=w_t["W_lin2"][:], rhs=a1[:],
                                 start=True, stop=True)
                a2 = selpool.tile([64, w], f32, tag="a2", name=f"a2_{c0}")
                nc.scalar.activation(a2[:], pa2[:], AF.Relu,
                                     bias=w_t["b_lin2"][:, 0:1])
                pa3 = psS.tile([1, w], f32, tag="s", name=f"pa3_{c0}")
                nc.tensor.matmul(pa3[:], lhsT=w_t["W_lin3"][:], rhs=a2[:],
                                 start=True, stop=True)
                a3 = selpool.tile([1, w], f32, tag="a3", name=f"a3_{c0}")
                nc.scalar.activation(a3[:], pa3[:], AF.Identity,
                                     bias=w_t["b_lin3"][:, 0:1])
                nc.sync.dma_start(out=d_out[:, sl], in_=a3[:])

            load_A(0)
            load_xT(0)
            load_xT(1)
            load_A(1)
            load_A(2)
            load_xT(2)
            load_xT(3)
            load_A(3)
            chains = [pair_chain(p) for p in range(G_PER_CORE // 2)]
            # start chain k+1 once chain k has advanced THRESH[k] yields
            THRESH = [11, 11, 6]
            progress = [0] * len(chains)
            done = [False] * len(chains)
            started = 1
            mlp_first_emitted = False
            while not all(done):
                for k in range(started):
                    if done[k]:
                        continue
                    try:
                        next(chains[k])
                        progress[k] += 1
                    except StopIteration:
                        done[k] = True
                        if done[0] and done[1] and not mlp_first_emitted:
                            mlp(0, 4)
                            mlp_first_emitted = True
                        if len(done) > 2 and done[2] and mlp_first_emitted \
                                and not getattr(mlp, "_mid", False):
                            mlp(4, 6)
                            mlp._mid = True
                    if (k == started - 1 and started < len(chains)
                            and progress[k] >= THRESH[k]):
                        for g in (2 * started + 2, 2 * started + 3):
                            if g < G_PER_CORE:
                                load_xT(g)
                                load_A(g)
                        started += 1
            mlp(6, G_PER_CORE)

            # (MLP emitted by the driver, split in two graph-halves)

    nc.compile()
    return nc


HALF512 = 512


def prepare_inputs(inputs):
    """Host index-preprocessing + sharding. Returns per-core input maps."""
    x = np.asarray(inputs["x"], np.float32)
    ei = np.asarray(inputs["edge_index"], np.int64)
    src = ei[0] % N
    dst = ei[1] % N
    gid = ei[0] // N

    maps = []
    for core in range(N_CORES):
        gs = range(core * G_PER_CORE, (core + 1) * G_PER_CORE)
        xs = np.empty((P, G_PER_CORE, NCH, IN_F), np.float32)
        xT = np.empty((G_PER_CORE, IN_F, N), np.float32)
        As = np.empty((G_PER_CORE, P, NCH * N), np.float32)
        for i, g in enumerate(gs):
            xg = x[g * N:(g + 1) * N]                       # [N, IN_F]
            xs[:, i] = xg.reshape(NCH, P, IN_F).transpose(1, 0, 2)
            xT[i] = xg.T
            m = gid == g
            A = np.zeros((N, N), np.float32)
            np.add.at(A, (src[m], dst[m]), 1.0)
            # device layout [p, (half, chunk, 512)]
            Ah = A.reshape(NCH, P, 2, HALF512).transpose(1, 2, 0, 3)
            As[i] = Ah.reshape(P, NCH * N)
        im = {"x_nm": xs.reshape(P, G_PER_CORE * NCH * IN_F), "A_sd": As,
              "xT": xT}
        for l in (1, 2, 3):
            im[f"W_rel{l}"] = np.asarray(inputs[f"W_rel{l}"], np.float32)
            im[f"W_root{l}"] = np.asarray(inputs[f"W_root{l}"], np.float32)
            im[f"b_rel{l}"] = np.asarray(inputs[f"b_rel{l}"], np.float32).reshape(HID, 1)
            im[f"w_pool{l}"] = np.asarray(inputs[f"w_pool{l}"], np.float32).reshape(HID, 1)
        W1 = np.asarray(inputs["W_lin1"], np.float32)
        im["W_lin1a"] = np.ascontiguousarray(W1[:HID])
        im["W_lin1b"] = np.ascontiguousarray(W1[HID:])
        im["b_lin1"] = np.asarray(inputs["b_lin1"], np.float32).reshape(HID, 1)
        im["W_lin2"] = np.asarray(inputs["W_lin2"], np.float32)
        im["b_lin2"] = np.asarray(inputs["b_lin2"], np.float32).reshape(64, 1)
        im["W_lin3"] = np.asarray(inputs["W_lin3"], np.float32)
        im["b_lin3"] = np.asarray(inputs["b_lin3"], np.float32).reshape(1, 1)
        maps.append(im)
    return maps


def run_on_device(inputs, trace=False):
    from concourse.bass_utils import run_bass_kernel_spmd
    nc = build_program()
    maps = prepare_inputs(inputs)
    res = run_bass_kernel_spmd(nc, maps, core_ids=list(range(N_CORES)),
                               trace=trace)
    outs = [res.results[c]["out"].reshape(-1) for c in range(N_CORES)]
    full = np.concatenate(outs).astype(np.float32).reshape(B_GRAPHS, 1)
    return full, res


def kernel(**inputs) -> np.ndarray:
    out, _ = run_on_device(inputs)
    return out
